# revision 7
# baseline (speedup 1.0000x reference)
"""Masked 5x5 group-causal conv (PixelCNN-style) + bias + per-channel PReLU.

Problem: x (8, 128, 256, 512) f32, weight (128, 128, 5, 5) f32 masked by a
fixed causal mask, SAME conv, + bias + PReLU.  The mask zeroes everything
below/right-of the causal frontier, leaving only 13 live taps:
  ky=0,1 (dy=-2,-1): all 5 kx;  ky=2 (dy=0): kx=0,1 (left) and the
  group-masked center tap (2,2).
The mask is constant, so it is folded into the weights on the host.

Sharding: data-parallel over batch — core i computes batch element i.

Per-core kernel: for each output row h, accumulate 13 matmuls
(cin=128 contraction on partitions, cout=128 stationary free dim,
W=512 moving free dim) into one PSUM bank, then one ScalarE Prelu
activation (fused +bias) drains PSUM -> SBUF, and batched DMAs move
rows HBM<->SBUF in 8-row bands.  Two matmul dtype variants:
  bf16   — inputs cast to bf16 on DVE (absmax rel err ~2.3e-3)
  fp32r  — fp32 data fed to the PE in float32r mode (1 cyc/row at N>=256)
Accumulation is f32 in PSUM either way.
"""

import numpy as np

B, C, H, W = 8, 128, 256, 512
KS = 5
PAD = 2
RB = 8  # rows per band (one PSUM bank per row)
NBANDS = H // RB

# 13 live taps (ky, kx) of the causal mask, in accumulation order.
TAPS = [(ky, kx) for ky in range(2) for kx in range(KS)] + [(2, 0), (2, 1), (2, 2)]
NT = len(TAPS)

NGROUP, CIN_G, COUT_G = 8, 16, 16


def _build_mask() -> np.ndarray:
    c = KS // 2
    m = np.zeros((C, C, KS, KS), dtype=np.float32)
    m[:, :, :c, :] = 1.0
    m[:, :, c, :c] = 1.0
    g_out = np.arange(C)[:, None] // COUT_G
    g_in = np.arange(C)[None, :] // CIN_G
    m[:, :, c, c] = (g_in <= g_out).astype(np.float32)  # hidden layer
    return m


_CACHE = {}


def _build_bass(dtype_tag: str):
    import concourse.bacc as bacc
    import concourse.mybir as mybir
    from concourse.tile import TileContext

    dt = mybir.dt
    mm_dt = dt.bfloat16 if dtype_tag == "bf16" else dt.float32r

    nc = bacc.Bacc("TRN2", target_bir_lowering=False)
    # fp32r variant: declare x as float32r (same bytes as f32) so the DMA
    # into the float32r SBUF band tile is not a cast.
    x_dt = dt.float32 if dtype_tag == "bf16" else dt.float32r
    x = nc.dram_tensor("x", [C, H, W], x_dt, kind="ExternalInput")
    w = nc.dram_tensor("w", [C, NT * C], mm_dt, kind="ExternalInput")
    bias = nc.dram_tensor("bias", [C, 1], dt.float32, kind="ExternalInput")
    slope = nc.dram_tensor("slope", [C, 1], dt.float32, kind="ExternalInput")
    y = nc.dram_tensor("y", [C, H, W], dt.float32, kind="ExternalOutput")

    WP = W + 2 * PAD  # padded row width

    with TileContext(nc) as tc:
        with (
            tc.tile_pool(name="const", bufs=1) as cpool,
            tc.tile_pool(name="xin", bufs=3) as xin_pool,
            tc.tile_pool(name="xband", bufs=4) as xband_pool,
            tc.tile_pool(name="oband", bufs=3) as out_pool,
            tc.tile_pool(name="ps", bufs=8, space="PSUM") as psum_pool,
        ):
            w_sb = cpool.tile([C, NT * C], mm_dt, name="w_sb")
            nc.sync.dma_start(w_sb[:, :], w[:, :])
            bias_sb = cpool.tile([C, 1], dt.float32, name="bias_sb")
            nc.sync.dma_start(bias_sb[:, :], bias[:, :])
            slope_sb = cpool.tile([C, 1], dt.float32, name="slope_sb")
            nc.sync.dma_start(slope_sb[:, :], slope[:, :])

            bands = {}  # band index -> (128, RB, WP) tile in mm_dt

            def load_band(b):
                h0 = b * RB
                xb = xband_pool.tile([C, RB, WP], mm_dt, name="xb")
                nc.gpsimd.memset(xb[:, :, 0:PAD], 0.0)
                nc.gpsimd.memset(xb[:, :, W + PAD : WP], 0.0)
                if dtype_tag == "bf16":
                    xin = xin_pool.tile([C, RB, W], dt.float32, name="xin")
                    nc.sync.dma_start(xin[:, :, :], x[:, h0 : h0 + RB, :])
                    nc.vector.tensor_copy(xb[:, :, PAD : W + PAD], xin[:, :, :])
                else:
                    nc.sync.dma_start(
                        xb[:, :, PAD : W + PAD], x[:, h0 : h0 + RB, :]
                    )
                bands[b] = xb

            def row_ap(h, dx):
                """(128, 512) moving operand for source row h shifted by dx."""
                b, r = divmod(h, RB)
                return bands[b][:, r, PAD + dx : PAD + dx + W]

            for b in range(NBANDS):
                if b == 0:
                    load_band(0)
                if b + 1 < NBANDS:
                    load_band(b + 1)  # prefetch
                h0 = b * RB
                psums = [psum_pool.tile([C, W], dt.float32, name="ps") for _ in range(RB)]
                # valid taps per row (rows 0/1 lose the dy=-2/-1 taps)
                valid = []
                for r in range(RB):
                    h = h0 + r
                    valid.append(
                        [t for t, (ky, kx) in enumerate(TAPS) if h + ky - PAD >= 0]
                    )
                # taps outer, rows inner: reuses each stationary weight 8x
                for t, (ky, kx) in enumerate(TAPS):
                    dy, dx = ky - PAD, kx - PAD
                    for r in range(RB):
                        h = h0 + r
                        if h + dy < 0:
                            continue
                        nc.tensor.matmul(
                            psums[r][:, :],
                            w_sb[:, t * C : (t + 1) * C],
                            row_ap(h + dy, dx),
                            start=(t == valid[r][0]),
                            stop=(t == valid[r][-1]),
                        )
                ob = out_pool.tile([C, RB, W], dt.float32, name="ob")
                for r in range(RB):
                    nc.scalar.activation(
                        ob[:, r, :],
                        psums[r][:, :],
                        mybir.ActivationFunctionType.Prelu,
                        bias=bias_sb[:, 0:1],
                        scale=1.0,
                        alpha=slope_sb[:, 0:1],
                    )
                nc.sync.dma_start(y[:, h0 : h0 + RB, :], ob[:, :, :])
                if b - 1 in bands:
                    del bands[b - 1]
    nc.compile()
    return nc


def _get_nc(dtype_tag: str):
    if dtype_tag not in _CACHE:
        _CACHE[dtype_tag] = _build_bass(dtype_tag)
    return _CACHE[dtype_tag]


def _prep_weights(weight: np.ndarray, dtype_tag: str) -> np.ndarray:
    wm = weight.astype(np.float32) * _build_mask()
    wt = np.transpose(wm, (2, 3, 1, 0))  # (ky, kx, cin, cout)
    w_taps = np.concatenate([wt[ky, kx] for ky, kx in TAPS], axis=1)  # (128, 13*128)
    if dtype_tag == "bf16":
        import ml_dtypes

        return np.ascontiguousarray(w_taps).astype(ml_dtypes.bfloat16)
    return np.ascontiguousarray(w_taps)


def kernel(x, weight, bias, slope, dtype_tag="bf16", trace=False):
    from concourse.bass_utils import run_bass_kernel_spmd

    nc = _get_nc(dtype_tag)
    w_in = _prep_weights(np.asarray(weight), dtype_tag)
    bias_in = np.ascontiguousarray(np.asarray(bias, dtype=np.float32).reshape(C, 1))
    slope_in = np.ascontiguousarray(np.asarray(slope, dtype=np.float32).reshape(C, 1))
    x = np.asarray(x, dtype=np.float32)
    in_maps = [
        {
            "x": np.ascontiguousarray(x[i]),
            "w": w_in,
            "bias": bias_in,
            "slope": slope_in,
        }
        for i in range(B)
    ]
    res = run_bass_kernel_spmd(nc, in_maps, core_ids=list(range(B)), trace=trace)
    y = np.stack([res.results[i]["y"] for i in range(B)], axis=0)
    if trace:
        return y, res
    return y


# revision 10
# speedup vs baseline: 1.0153x; 1.0153x over previous
"""Masked 5x5 group-causal conv (PixelCNN-style) + bias + per-channel PReLU.

Problem: x (8, 128, 256, 512) f32, weight (128, 128, 5, 5) f32 masked by a
fixed causal mask, SAME conv, + bias + PReLU.  The mask zeroes everything
below/right-of the causal frontier, leaving only 13 live taps:
  ky=0,1 (dy=-2,-1): all 5 kx;  ky=2 (dy=0): kx=0,1 (left) and the
  group-masked center tap (2,2).
The mask is constant, so it is folded into the weights on the host.

Sharding: data-parallel over batch — core i computes batch element i.

Per-core kernel: for each output row h, accumulate 13 matmuls
(cin=128 contraction on partitions, cout=128 stationary free dim,
W=512 moving free dim) into one PSUM bank, then one ScalarE Prelu
activation (fused +bias) drains PSUM -> SBUF, and batched DMAs move
rows HBM<->SBUF in 8-row bands.  Two matmul dtype variants:
  bf16   — inputs cast to bf16 on DVE (absmax rel err ~2.3e-3)
  fp32r  — fp32 data fed to the PE in float32r mode (1 cyc/row at N>=256)
Accumulation is f32 in PSUM either way.
"""

import numpy as np

B, C, H, W = 8, 128, 256, 512
KS = 5
PAD = 2
RB = 8  # rows per band (one PSUM bank per row)
NBANDS = H // RB

# 13 live taps (ky, kx) of the causal mask, in accumulation order.
TAPS = [(ky, kx) for ky in range(2) for kx in range(KS)] + [(2, 0), (2, 1), (2, 2)]
NT = len(TAPS)

NGROUP, CIN_G, COUT_G = 8, 16, 16


def _build_mask() -> np.ndarray:
    c = KS // 2
    m = np.zeros((C, C, KS, KS), dtype=np.float32)
    m[:, :, :c, :] = 1.0
    m[:, :, c, :c] = 1.0
    g_out = np.arange(C)[:, None] // COUT_G
    g_in = np.arange(C)[None, :] // CIN_G
    m[:, :, c, c] = (g_in <= g_out).astype(np.float32)  # hidden layer
    return m


_CACHE = {}


def _build_bass(dtype_tag: str):
    import concourse.bacc as bacc
    import concourse.mybir as mybir
    from concourse.tile import TileContext

    dt = mybir.dt
    mm_dt = dt.bfloat16 if dtype_tag == "bf16" else dt.float32r

    nc = bacc.Bacc("TRN2", target_bir_lowering=False)
    # fp32r variant: declare x as float32r (same bytes as f32) so the DMA
    # into the float32r SBUF band tile is not a cast.
    x_dt = dt.float32 if dtype_tag == "bf16" else dt.float32r
    x = nc.dram_tensor("x", [C, H, W], x_dt, kind="ExternalInput")
    w = nc.dram_tensor("w", [C, NT * C], mm_dt, kind="ExternalInput")
    bias = nc.dram_tensor("bias", [C, 1], dt.float32, kind="ExternalInput")
    slope = nc.dram_tensor("slope", [C, 1], dt.float32, kind="ExternalInput")
    y = nc.dram_tensor("y", [C, H, W], dt.float32, kind="ExternalOutput")

    WP = W + 2 * PAD  # padded row width

    with TileContext(nc) as tc:
        with (
            tc.tile_pool(name="const", bufs=1) as cpool,
            tc.tile_pool(name="xin", bufs=3) as xin_pool,
            tc.tile_pool(name="xband", bufs=4) as xband_pool,
            tc.tile_pool(name="oband", bufs=3) as out_pool,
            tc.tile_pool(name="ps", bufs=8, space="PSUM") as psum_pool,
        ):
            w_sb = cpool.tile([C, NT * C], mm_dt, name="w_sb")
            nc.sync.dma_start(w_sb[:, :], w[:, :])
            bias_sb = cpool.tile([C, 1], dt.float32, name="bias_sb")
            nc.sync.dma_start(bias_sb[:, :], bias[:, :])
            slope_sb = cpool.tile([C, 1], dt.float32, name="slope_sb")
            nc.sync.dma_start(slope_sb[:, :], slope[:, :])

            bands = {}  # band index -> (128, RB, WP) tile in mm_dt

            def load_band(b, per_row=False):
                h0 = b * RB
                xb = xband_pool.tile([C, RB, WP], mm_dt, name="xb")
                nc.gpsimd.memset(xb[:, :, 0:PAD], 0.0)
                nc.gpsimd.memset(xb[:, :, W + PAD : WP], 0.0)
                # per_row (band 0): row-granular load+cast so the first
                # matmuls start after one 256KB row, not the whole 2MB band.
                chunks = (
                    [(r, 1) for r in range(RB)] if per_row else [(0, RB)]
                )
                if dtype_tag == "bf16":
                    xin = xin_pool.tile([C, RB, W], dt.float32, name="xin")
                    for r0, nr in chunks:
                        nc.sync.dma_start(
                            xin[:, r0 : r0 + nr, :], x[:, h0 + r0 : h0 + r0 + nr, :]
                        )
                        nc.vector.tensor_copy(
                            xb[:, r0 : r0 + nr, PAD : W + PAD], xin[:, r0 : r0 + nr, :]
                        )
                else:
                    for r0, nr in chunks:
                        nc.sync.dma_start(
                            xb[:, r0 : r0 + nr, PAD : W + PAD],
                            x[:, h0 + r0 : h0 + r0 + nr, :],
                        )
                bands[b] = xb

            def row_ap(h, dx):
                """(128, 512) moving operand for source row h shifted by dx."""
                b, r = divmod(h, RB)
                return bands[b][:, r, PAD + dx : PAD + dx + W]

            for b in range(NBANDS):
                if b == 0:
                    load_band(0, per_row=True)
                if b + 1 < NBANDS:
                    load_band(b + 1)  # prefetch
                h0 = b * RB
                psums = [psum_pool.tile([C, W], dt.float32, name="ps") for _ in range(RB)]
                # valid taps per row (rows 0/1 lose the dy=-2/-1 taps)
                valid = []
                for r in range(RB):
                    h = h0 + r
                    valid.append(
                        [t for t, (ky, kx) in enumerate(TAPS) if h + ky - PAD >= 0]
                    )
                # taps outer, rows inner: reuses each stationary weight 8x
                for t, (ky, kx) in enumerate(TAPS):
                    dy, dx = ky - PAD, kx - PAD
                    for r in range(RB):
                        h = h0 + r
                        if h + dy < 0:
                            continue
                        nc.tensor.matmul(
                            psums[r][:, :],
                            w_sb[:, t * C : (t + 1) * C],
                            row_ap(h + dy, dx),
                            start=(t == valid[r][0]),
                            stop=(t == valid[r][-1]),
                        )
                ob = out_pool.tile([C, RB, W], dt.float32, name="ob")
                for r in range(RB):
                    nc.scalar.activation(
                        ob[:, r, :],
                        psums[r][:, :],
                        mybir.ActivationFunctionType.Prelu,
                        bias=bias_sb[:, 0:1],
                        scale=1.0,
                        alpha=slope_sb[:, 0:1],
                    )
                if b == NBANDS - 1:
                    # last band: drain output progressively behind the ACTs
                    for r0 in range(0, RB, 2):
                        nc.sync.dma_start(
                            y[:, h0 + r0 : h0 + r0 + 2, :], ob[:, r0 : r0 + 2, :]
                        )
                else:
                    nc.sync.dma_start(y[:, h0 : h0 + RB, :], ob[:, :, :])
                if b - 1 in bands:
                    del bands[b - 1]
    nc.compile()
    return nc


def _get_nc(dtype_tag: str):
    if dtype_tag not in _CACHE:
        _CACHE[dtype_tag] = _build_bass(dtype_tag)
    return _CACHE[dtype_tag]


def _prep_weights(weight: np.ndarray, dtype_tag: str) -> np.ndarray:
    wm = weight.astype(np.float32) * _build_mask()
    wt = np.transpose(wm, (2, 3, 1, 0))  # (ky, kx, cin, cout)
    w_taps = np.concatenate([wt[ky, kx] for ky, kx in TAPS], axis=1)  # (128, 13*128)
    if dtype_tag == "bf16":
        import ml_dtypes

        return np.ascontiguousarray(w_taps).astype(ml_dtypes.bfloat16)
    return np.ascontiguousarray(w_taps)


def kernel(x, weight, bias, slope, dtype_tag="bf16", trace=False):
    from concourse.bass_utils import run_bass_kernel_spmd

    nc = _get_nc(dtype_tag)
    w_in = _prep_weights(np.asarray(weight), dtype_tag)
    bias_in = np.ascontiguousarray(np.asarray(bias, dtype=np.float32).reshape(C, 1))
    slope_in = np.ascontiguousarray(np.asarray(slope, dtype=np.float32).reshape(C, 1))
    x = np.asarray(x, dtype=np.float32)
    in_maps = [
        {
            "x": np.ascontiguousarray(x[i]),
            "w": w_in,
            "bias": bias_in,
            "slope": slope_in,
        }
        for i in range(B)
    ]
    res = run_bass_kernel_spmd(nc, in_maps, core_ids=list(range(B)), trace=trace)
    y = np.stack([res.results[i]["y"] for i in range(B)], axis=0)
    if trace:
        return y, res
    return y


# revision 12
# speedup vs baseline: 1.0429x; 1.0272x over previous
"""Masked 5x5 group-causal conv (PixelCNN-style) + bias + per-channel PReLU.

Problem: x (8, 128, 256, 512) f32, weight (128, 128, 5, 5) f32 masked by a
fixed causal mask, SAME conv, + bias + PReLU.  The mask zeroes everything
below/right-of the causal frontier, leaving only 13 live taps:
  ky=0,1 (dy=-2,-1): all 5 kx;  ky=2 (dy=0): kx=0,1 (left) and the
  group-masked center tap (2,2).
The mask is constant, so it is folded into the weights on the host.

Sharding: data-parallel over batch — core i computes batch element i.

Per-core kernel: for each output row h, accumulate 13 matmuls
(cin=128 contraction on partitions, cout=128 stationary free dim,
W=512 moving free dim) into one PSUM bank, then one ScalarE Prelu
activation (fused +bias) drains PSUM -> SBUF, and batched DMAs move
rows HBM<->SBUF in 8-row bands.  Two matmul dtype variants:
  bf16   — inputs cast to bf16 on DVE (absmax rel err ~2.3e-3)
  fp32r  — fp32 data fed to the PE in float32r mode (1 cyc/row at N>=256)
Accumulation is f32 in PSUM either way.
"""

import numpy as np

B, C, H, W = 8, 128, 256, 512
KS = 5
PAD = 2
RB = 8  # rows per band (one PSUM bank per row)
NBANDS = H // RB

# 13 live taps (ky, kx) of the causal mask, in accumulation order.
TAPS = [(ky, kx) for ky in range(2) for kx in range(KS)] + [(2, 0), (2, 1), (2, 2)]
NT = len(TAPS)

NGROUP, CIN_G, COUT_G = 8, 16, 16


def _build_mask() -> np.ndarray:
    c = KS // 2
    m = np.zeros((C, C, KS, KS), dtype=np.float32)
    m[:, :, :c, :] = 1.0
    m[:, :, c, :c] = 1.0
    g_out = np.arange(C)[:, None] // COUT_G
    g_in = np.arange(C)[None, :] // CIN_G
    m[:, :, c, c] = (g_in <= g_out).astype(np.float32)  # hidden layer
    return m


_CACHE = {}


def _build_bass(dtype_tag: str):
    import concourse.bacc as bacc
    import concourse.mybir as mybir
    from concourse.tile import TileContext

    dt = mybir.dt
    mm_dt = dt.bfloat16 if dtype_tag == "bf16" else dt.float32r

    nc = bacc.Bacc("TRN2", target_bir_lowering=False)
    # fp32r variant: declare x as float32r (same bytes as f32) so the DMA
    # into the float32r SBUF band tile is not a cast.
    x_dt = dt.float32 if dtype_tag == "bf16" else dt.float32r
    x = nc.dram_tensor("x", [C, H, W], x_dt, kind="ExternalInput")
    w = nc.dram_tensor("w", [C, NT * C], mm_dt, kind="ExternalInput")
    bias = nc.dram_tensor("bias", [C, 1], dt.float32, kind="ExternalInput")
    slope = nc.dram_tensor("slope", [C, 1], dt.float32, kind="ExternalInput")
    y = nc.dram_tensor("y", [C, H, W], dt.float32, kind="ExternalOutput")

    WP = W + 2 * PAD  # padded row width

    with TileContext(nc) as tc:
        with (
            tc.tile_pool(name="const", bufs=1) as cpool,
            tc.tile_pool(name="xin", bufs=3) as xin_pool,
            tc.tile_pool(name="xband", bufs=4) as xband_pool,
            tc.tile_pool(name="oband", bufs=3) as out_pool,
            tc.tile_pool(name="ps", bufs=8, space="PSUM") as psum_pool,
        ):
            w_sb = cpool.tile([C, NT * C], mm_dt, name="w_sb")
            nc.sync.dma_start(w_sb[:, :], w[:, :])
            bias_sb = cpool.tile([C, 1], dt.float32, name="bias_sb")
            nc.sync.dma_start(bias_sb[:, :], bias[:, :])
            slope_sb = cpool.tile([C, 1], dt.float32, name="slope_sb")
            nc.sync.dma_start(slope_sb[:, :], slope[:, :])

            bands = {}  # band index -> (128, RB, WP) tile in mm_dt

            def load_band(b, per_row=False):
                h0 = b * RB
                xb = xband_pool.tile([C, RB, WP], mm_dt, name="xb")
                nc.gpsimd.memset(xb[:, :, 0:PAD], 0.0)
                nc.gpsimd.memset(xb[:, :, W + PAD : WP], 0.0)
                # per_row (band 0): row-granular load+cast so the first
                # matmuls start after one 256KB row, not the whole 2MB band.
                chunks = (
                    [(r, 1) for r in range(RB)] if per_row else [(0, RB)]
                )
                if dtype_tag == "bf16":
                    xin = xin_pool.tile([C, RB, W], dt.float32, name="xin")
                    for r0, nr in chunks:
                        nc.sync.dma_start(
                            xin[:, r0 : r0 + nr, :], x[:, h0 + r0 : h0 + r0 + nr, :]
                        )
                        nc.vector.tensor_copy(
                            xb[:, r0 : r0 + nr, PAD : W + PAD], xin[:, r0 : r0 + nr, :]
                        )
                else:
                    for r0, nr in chunks:
                        nc.sync.dma_start(
                            xb[:, r0 : r0 + nr, PAD : W + PAD],
                            x[:, h0 + r0 : h0 + r0 + nr, :],
                        )
                bands[b] = xb

            def row_ap(h, dx):
                """(128, 512) moving operand for source row h shifted by dx."""
                b, r = divmod(h, RB)
                return bands[b][:, r, PAD + dx : PAD + dx + W]

            for b in range(NBANDS):
                if b == 0:
                    load_band(0, per_row=True)
                if b + 1 < NBANDS:
                    load_band(b + 1)  # prefetch
                h0 = b * RB
                psums = [psum_pool.tile([C, W], dt.float32, name="ps") for _ in range(RB)]
                # valid taps per row (rows 0/1 lose the dy=-2/-1 taps)
                valid = []
                for r in range(RB):
                    h = h0 + r
                    valid.append(
                        [t for t, (ky, kx) in enumerate(TAPS) if h + ky - PAD >= 0]
                    )
                # bank-major: all taps of a row consecutively, so each PSUM
                # bank stops ~13 MMs apart — its ACT drains while later rows
                # still matmul, and the next band never waits on bank release.
                # (Every matmul carries its own LDWEIGHTS either way, so tap
                # order costs nothing on the PE.)
                ob = out_pool.tile([C, RB, W], dt.float32, name="ob")
                for r in range(RB):
                    h = h0 + r
                    for t in valid[r]:
                        ky, kx = TAPS[t]
                        dy, dx = ky - PAD, kx - PAD
                        nc.tensor.matmul(
                            psums[r][:, :],
                            w_sb[:, t * C : (t + 1) * C],
                            row_ap(h + dy, dx),
                            start=(t == valid[r][0]),
                            stop=(t == valid[r][-1]),
                        )
                    nc.scalar.activation(
                        ob[:, r, :],
                        psums[r][:, :],
                        mybir.ActivationFunctionType.Prelu,
                        bias=bias_sb[:, 0:1],
                        scale=1.0,
                        alpha=slope_sb[:, 0:1],
                    )
                if b == NBANDS - 1:
                    # last band: drain output progressively behind the ACTs
                    for r0 in range(0, RB, 2):
                        nc.sync.dma_start(
                            y[:, h0 + r0 : h0 + r0 + 2, :], ob[:, r0 : r0 + 2, :]
                        )
                else:
                    nc.sync.dma_start(y[:, h0 : h0 + RB, :], ob[:, :, :])
                if b - 1 in bands:
                    del bands[b - 1]
    nc.compile()
    return nc


def _get_nc(dtype_tag: str):
    if dtype_tag not in _CACHE:
        _CACHE[dtype_tag] = _build_bass(dtype_tag)
    return _CACHE[dtype_tag]


def _prep_weights(weight: np.ndarray, dtype_tag: str) -> np.ndarray:
    wm = weight.astype(np.float32) * _build_mask()
    wt = np.transpose(wm, (2, 3, 1, 0))  # (ky, kx, cin, cout)
    w_taps = np.concatenate([wt[ky, kx] for ky, kx in TAPS], axis=1)  # (128, 13*128)
    if dtype_tag == "bf16":
        import ml_dtypes

        return np.ascontiguousarray(w_taps).astype(ml_dtypes.bfloat16)
    return np.ascontiguousarray(w_taps)


def kernel(x, weight, bias, slope, dtype_tag="bf16", trace=False):
    from concourse.bass_utils import run_bass_kernel_spmd

    nc = _get_nc(dtype_tag)
    w_in = _prep_weights(np.asarray(weight), dtype_tag)
    bias_in = np.ascontiguousarray(np.asarray(bias, dtype=np.float32).reshape(C, 1))
    slope_in = np.ascontiguousarray(np.asarray(slope, dtype=np.float32).reshape(C, 1))
    x = np.asarray(x, dtype=np.float32)
    in_maps = [
        {
            "x": np.ascontiguousarray(x[i]),
            "w": w_in,
            "bias": bias_in,
            "slope": slope_in,
        }
        for i in range(B)
    ]
    res = run_bass_kernel_spmd(nc, in_maps, core_ids=list(range(B)), trace=trace)
    y = np.stack([res.results[i]["y"] for i in range(B)], axis=0)
    if trace:
        return y, res
    return y


# revision 15
# speedup vs baseline: 1.0434x; 1.0005x over previous
"""Masked 5x5 group-causal conv (PixelCNN-style) + bias + per-channel PReLU.

Problem: x (8, 128, 256, 512) f32, weight (128, 128, 5, 5) f32 masked by a
fixed causal mask, SAME conv, + bias + PReLU.  The mask zeroes everything
below/right-of the causal frontier, leaving only 13 live taps:
  ky=0,1 (dy=-2,-1): all 5 kx;  ky=2 (dy=0): kx=0,1 (left) and the
  group-masked center tap (2,2).
The mask is constant, so it is folded into the weights on the host.

Sharding: data-parallel over batch — core i computes batch element i.

Per-core kernel: for each output row h, accumulate 13 matmuls
(cin=128 contraction on partitions, cout=128 stationary free dim,
W=512 moving free dim) into one PSUM bank, then one ScalarE Prelu
activation (fused +bias) drains PSUM -> SBUF, and batched DMAs move
rows HBM<->SBUF in 8-row bands.  Two matmul dtype variants:
  bf16   — inputs cast to bf16 on DVE (absmax rel err ~2.3e-3)
  fp32r  — fp32 data fed to the PE in float32r mode (1 cyc/row at N>=256)
Accumulation is f32 in PSUM either way.
"""

import numpy as np

B, C, H, W = 8, 128, 256, 512
KS = 5
PAD = 2
RB = 8  # rows per band (one PSUM bank per row)
NBANDS = H // RB

# 13 live taps (ky, kx) of the causal mask, in accumulation order.
TAPS = [(ky, kx) for ky in range(2) for kx in range(KS)] + [(2, 0), (2, 1), (2, 2)]
NT = len(TAPS)

NGROUP, CIN_G, COUT_G = 8, 16, 16


def _build_mask() -> np.ndarray:
    c = KS // 2
    m = np.zeros((C, C, KS, KS), dtype=np.float32)
    m[:, :, :c, :] = 1.0
    m[:, :, c, :c] = 1.0
    g_out = np.arange(C)[:, None] // COUT_G
    g_in = np.arange(C)[None, :] // CIN_G
    m[:, :, c, c] = (g_in <= g_out).astype(np.float32)  # hidden layer
    return m


_CACHE = {}


def _build_bass(dtype_tag: str):
    import concourse.bacc as bacc
    import concourse.mybir as mybir
    from concourse.tile import TileContext

    dt = mybir.dt
    mm_dt = dt.bfloat16 if dtype_tag == "bf16" else dt.float32r

    nc = bacc.Bacc("TRN2", target_bir_lowering=False)
    # fp32r variant: declare x as float32r (same bytes as f32) so the DMA
    # into the float32r SBUF band tile is not a cast.
    x_dt = dt.float32 if dtype_tag == "bf16" else dt.float32r
    x = nc.dram_tensor("x", [C, H, W], x_dt, kind="ExternalInput")
    w = nc.dram_tensor("w", [C, NT * C], mm_dt, kind="ExternalInput")
    bias = nc.dram_tensor("bias", [C, 1], dt.float32, kind="ExternalInput")
    slope = nc.dram_tensor("slope", [C, 1], dt.float32, kind="ExternalInput")
    y = nc.dram_tensor("y", [C, H, W], dt.float32, kind="ExternalOutput")

    WP = W + 2 * PAD  # padded row width

    with TileContext(nc) as tc:
        with (
            tc.tile_pool(name="const", bufs=1) as cpool,
            tc.tile_pool(name="xin", bufs=3) as xin_pool,
            tc.tile_pool(name="xband", bufs=4) as xband_pool,
            tc.tile_pool(name="oband", bufs=3) as out_pool,
            tc.tile_pool(name="ps", bufs=8, space="PSUM") as psum_pool,
        ):
            bands = {}  # band index -> (128, RB, WP) tile in mm_dt

            def load_band(b, chunks=((0, RB),)):
                h0 = b * RB
                xb = xband_pool.tile([C, RB, WP], mm_dt, name="xb")
                nc.gpsimd.memset(xb[:, :, 0:PAD], 0.0)
                nc.gpsimd.memset(xb[:, :, W + PAD : WP], 0.0)
                if dtype_tag == "bf16":
                    xin = xin_pool.tile([C, RB, W], dt.float32, name="xin")
                    for r0, nr in chunks:
                        nc.sync.dma_start(
                            xin[:, r0 : r0 + nr, :], x[:, h0 + r0 : h0 + r0 + nr, :]
                        )
                        nc.vector.tensor_copy(
                            xb[:, r0 : r0 + nr, PAD : W + PAD], xin[:, r0 : r0 + nr, :]
                        )
                else:
                    for r0, nr in chunks:
                        nc.sync.dma_start(
                            xb[:, r0 : r0 + nr, PAD : W + PAD],
                            x[:, h0 + r0 : h0 + r0 + nr, :],
                        )
                bands[b] = xb

            # Startup ordering: row 0 unlocks the first matmuls, so its DMA
            # trigger goes first, then the weights (transfer in parallel on
            # another queue), then the rest of band 0; bias/slope are only
            # needed by the first ACT (~16us in).
            xb0 = xband_pool.tile([C, RB, WP], mm_dt, name="xb")
            nc.gpsimd.memset(xb0[:, :, 0:PAD], 0.0)
            nc.gpsimd.memset(xb0[:, :, W + PAD : WP], 0.0)
            xin0 = xin_pool.tile([C, RB, W], dt.float32, name="xin")
            w_sb = cpool.tile([C, NT * C], mm_dt, name="w_sb")
            b0_chunks = [(0, 1), (1, 1), (2, 2), (4, 2), (6, 2)]
            for k, (r0, nr) in enumerate(b0_chunks):
                if dtype_tag == "bf16":
                    nc.sync.dma_start(xin0[:, r0 : r0 + nr, :], x[:, r0 : r0 + nr, :])
                    nc.vector.tensor_copy(
                        xb0[:, r0 : r0 + nr, PAD : W + PAD], xin0[:, r0 : r0 + nr, :]
                    )
                else:
                    nc.sync.dma_start(
                        xb0[:, r0 : r0 + nr, PAD : W + PAD], x[:, r0 : r0 + nr, :]
                    )
                if k == 0:
                    nc.sync.dma_start(w_sb[:, :], w[:, :])
            bands[0] = xb0
            bias_sb = cpool.tile([C, 1], dt.float32, name="bias_sb")
            nc.sync.dma_start(bias_sb[:, :], bias[:, :])
            slope_sb = cpool.tile([C, 1], dt.float32, name="slope_sb")
            nc.sync.dma_start(slope_sb[:, :], slope[:, :])

            def row_ap(h, dx):
                """(128, 512) moving operand for source row h shifted by dx."""
                b, r = divmod(h, RB)
                return bands[b][:, r, PAD + dx : PAD + dx + W]

            for b in range(NBANDS):
                if b + 1 < NBANDS:
                    load_band(b + 1)  # prefetch
                h0 = b * RB
                psums = [psum_pool.tile([C, W], dt.float32, name="ps") for _ in range(RB)]
                # valid taps per row (rows 0/1 lose the dy=-2/-1 taps)
                valid = []
                for r in range(RB):
                    h = h0 + r
                    valid.append(
                        [t for t, (ky, kx) in enumerate(TAPS) if h + ky - PAD >= 0]
                    )
                # bank-major: all taps of a row consecutively, so each PSUM
                # bank stops ~13 MMs apart — its ACT drains while later rows
                # still matmul, and the next band never waits on bank release.
                # (Every matmul carries its own LDWEIGHTS either way, so tap
                # order costs nothing on the PE.)
                ob = out_pool.tile([C, RB, W], dt.float32, name="ob")
                for r in range(RB):
                    h = h0 + r
                    for t in valid[r]:
                        ky, kx = TAPS[t]
                        dy, dx = ky - PAD, kx - PAD
                        nc.tensor.matmul(
                            psums[r][:, :],
                            w_sb[:, t * C : (t + 1) * C],
                            row_ap(h + dy, dx),
                            start=(t == valid[r][0]),
                            stop=(t == valid[r][-1]),
                        )
                    nc.scalar.activation(
                        ob[:, r, :],
                        psums[r][:, :],
                        mybir.ActivationFunctionType.Prelu,
                        bias=bias_sb[:, 0:1],
                        scale=1.0,
                        alpha=slope_sb[:, 0:1],
                    )
                if b == NBANDS - 1:
                    # last band: drain output progressively behind the ACTs
                    for r0 in range(0, RB, 2):
                        nc.sync.dma_start(
                            y[:, h0 + r0 : h0 + r0 + 2, :], ob[:, r0 : r0 + 2, :]
                        )
                else:
                    nc.sync.dma_start(y[:, h0 : h0 + RB, :], ob[:, :, :])
                if b - 1 in bands:
                    del bands[b - 1]
    nc.compile()
    return nc


def _get_nc(dtype_tag: str):
    if dtype_tag not in _CACHE:
        _CACHE[dtype_tag] = _build_bass(dtype_tag)
    return _CACHE[dtype_tag]


def _prep_weights(weight: np.ndarray, dtype_tag: str) -> np.ndarray:
    wm = weight.astype(np.float32) * _build_mask()
    wt = np.transpose(wm, (2, 3, 1, 0))  # (ky, kx, cin, cout)
    w_taps = np.concatenate([wt[ky, kx] for ky, kx in TAPS], axis=1)  # (128, 13*128)
    if dtype_tag == "bf16":
        import ml_dtypes

        return np.ascontiguousarray(w_taps).astype(ml_dtypes.bfloat16)
    return np.ascontiguousarray(w_taps)


def kernel(x, weight, bias, slope, dtype_tag="bf16", trace=False):
    from concourse.bass_utils import run_bass_kernel_spmd

    nc = _get_nc(dtype_tag)
    w_in = _prep_weights(np.asarray(weight), dtype_tag)
    bias_in = np.ascontiguousarray(np.asarray(bias, dtype=np.float32).reshape(C, 1))
    slope_in = np.ascontiguousarray(np.asarray(slope, dtype=np.float32).reshape(C, 1))
    x = np.asarray(x, dtype=np.float32)
    in_maps = [
        {
            "x": np.ascontiguousarray(x[i]),
            "w": w_in,
            "bias": bias_in,
            "slope": slope_in,
        }
        for i in range(B)
    ]
    res = run_bass_kernel_spmd(nc, in_maps, core_ids=list(range(B)), trace=trace)
    y = np.stack([res.results[i]["y"] for i in range(B)], axis=0)
    if trace:
        return y, res
    return y


# revision 28
# speedup vs baseline: 1.0462x; 1.0026x over previous
"""Masked 5x5 group-causal conv (PixelCNN-style) + bias + per-channel PReLU.

Problem: x (8, 128, 256, 512) f32, weight (128, 128, 5, 5) f32 masked by a
fixed causal mask, SAME conv, + bias + PReLU.  The mask zeroes everything
below/right-of the causal frontier, leaving only 13 live taps:
  ky=0,1 (dy=-2,-1): all 5 kx;  ky=2 (dy=0): kx=0,1 (left) and the
  group-masked center tap (2,2).
The mask is constant, so it is folded into the weights on the host.

Sharding: data-parallel over batch — core i computes batch element i.

Per-core kernel: for each output row h, accumulate 13 matmuls
(cin=128 contraction on partitions, cout=128 stationary free dim,
W=512 moving free dim) into one PSUM bank, then one ScalarE Prelu
activation (fused +bias) drains PSUM -> SBUF, and batched DMAs move
rows HBM<->SBUF in 8-row bands.  Matmul dtype variants (all 1 cyc/row
on the PE, f32 accumulation in PSUM):
  fp16 (default) — inputs cast on DVE; absmax rel err ~2.9e-4
  bf16           — absmax rel err ~2.2e-3
  fp32r          — fp32 bits in float32r mode (untested path)
"""

import numpy as np

B, C, H, W = 8, 128, 256, 512
KS = 5
PAD = 2
RB = 8  # rows per band (one PSUM bank per row)
NBANDS = H // RB

# 13 live taps (ky, kx) of the causal mask, in accumulation order.
TAPS = [(ky, kx) for ky in range(2) for kx in range(KS)] + [(2, 0), (2, 1), (2, 2)]
NT = len(TAPS)

NGROUP, CIN_G, COUT_G = 8, 16, 16


def _build_mask() -> np.ndarray:
    c = KS // 2
    m = np.zeros((C, C, KS, KS), dtype=np.float32)
    m[:, :, :c, :] = 1.0
    m[:, :, c, :c] = 1.0
    g_out = np.arange(C)[:, None] // COUT_G
    g_in = np.arange(C)[None, :] // CIN_G
    m[:, :, c, c] = (g_in <= g_out).astype(np.float32)  # hidden layer
    return m


_CACHE = {}


def _build_bass(dtype_tag: str):
    import concourse.bacc as bacc
    import concourse.mybir as mybir
    from concourse.tile import TileContext

    dt = mybir.dt
    mm_dt = {"bf16": dt.bfloat16, "fp16": dt.float16}.get(dtype_tag, dt.float32r)

    nc = bacc.Bacc("TRN2", target_bir_lowering=False)
    # fp32r variant: declare x as float32r (same bytes as f32) so the DMA
    # into the float32r SBUF band tile is not a cast.
    x_dt = dt.float32r if dtype_tag == "fp32r" else dt.float32
    x = nc.dram_tensor("x", [C, H, W], x_dt, kind="ExternalInput")
    w = nc.dram_tensor("w", [C, NT * C], mm_dt, kind="ExternalInput")
    bias = nc.dram_tensor("bias", [C, 1], dt.float32, kind="ExternalInput")
    slope = nc.dram_tensor("slope", [C, 1], dt.float32, kind="ExternalInput")
    y = nc.dram_tensor("y", [C, H, W], dt.float32, kind="ExternalOutput")

    WP = W + 2 * PAD  # padded row width

    with TileContext(nc) as tc:
        with (
            tc.tile_pool(name="const", bufs=1) as cpool,
            tc.tile_pool(name="xin", bufs=3) as xin_pool,
            tc.tile_pool(name="xband", bufs=5) as xband_pool,
            tc.tile_pool(name="oband", bufs=4) as out_pool,
            tc.tile_pool(name="ps", bufs=8, space="PSUM") as psum_pool,
        ):
            # PE warm-up: the HAM clock gate holds the PE at 1.2 GHz until
            # ~3.4us of sustained activity. Burn dummy matmuls on a zeroed
            # tile during the startup DMA window so the real stream starts
            # at 2.4 GHz.
            warm = cpool.tile([C, W], mm_dt, name="warm")
            nc.gpsimd.memset(warm[:, :], 0.0)
            ps_warm = psum_pool.tile([C, W], dt.float32, name="ps")
            for _ in range(7):
                nc.tensor.matmul(
                    ps_warm[:, :], warm[:, 0:C], warm[:, :], start=True, stop=True
                )

            bands = {}  # band index -> (128, RB, WP) tile in mm_dt

            def load_band(b, chunks=((0, RB // 2), (RB // 2, RB // 2))):
                h0 = b * RB
                xb = xband_pool.tile([C, RB, WP], mm_dt, name="xb")
                nc.gpsimd.memset(xb[:, :, 0:PAD], 0.0)
                nc.gpsimd.memset(xb[:, :, W + PAD : WP], 0.0)
                if dtype_tag != "fp32r":
                    xin = xin_pool.tile([C, RB, W], dt.float32, name="xin")
                    for r0, nr in chunks:
                        nc.sync.dma_start(
                            xin[:, r0 : r0 + nr, :], x[:, h0 + r0 : h0 + r0 + nr, :]
                        )
                        nc.vector.tensor_copy(
                            xb[:, r0 : r0 + nr, PAD : W + PAD], xin[:, r0 : r0 + nr, :]
                        )
                else:
                    for r0, nr in chunks:
                        nc.sync.dma_start(
                            xb[:, r0 : r0 + nr, PAD : W + PAD],
                            x[:, h0 + r0 : h0 + r0 + nr, :],
                        )
                bands[b] = xb

            # Startup ordering: row 0 unlocks the first matmuls, so its DMA
            # trigger goes first, then the weights (transfer in parallel on
            # another queue), then the rest of band 0; bias/slope are only
            # needed by the first ACT (~16us in).
            xb0 = xband_pool.tile([C, RB, WP], mm_dt, name="xb")
            nc.gpsimd.memset(xb0[:, :, 0:PAD], 0.0)
            nc.gpsimd.memset(xb0[:, :, W + PAD : WP], 0.0)
            xin0 = xin_pool.tile([C, RB, W], dt.float32, name="xin")
            w_sb = cpool.tile([C, NT * C], mm_dt, name="w_sb")
            b0_chunks = [(0, 1), (1, 1), (2, 2), (4, 2), (6, 2)]
            for k, (r0, nr) in enumerate(b0_chunks):
                if dtype_tag != "fp32r":
                    nc.sync.dma_start(xin0[:, r0 : r0 + nr, :], x[:, r0 : r0 + nr, :])
                    nc.vector.tensor_copy(
                        xb0[:, r0 : r0 + nr, PAD : W + PAD], xin0[:, r0 : r0 + nr, :]
                    )
                else:
                    nc.sync.dma_start(
                        xb0[:, r0 : r0 + nr, PAD : W + PAD], x[:, r0 : r0 + nr, :]
                    )
                if k == 0:
                    # rows 0/1 only need the dy=0 taps (10..12) — load those
                    # 192KB first so the first matmuls aren't gated on the
                    # full 832KB weight transfer.
                    nc.sync.dma_start(w_sb[:, 10 * C :], w[:, 10 * C :])
                elif k == 1:
                    nc.sync.dma_start(w_sb[:, : 10 * C], w[:, : 10 * C])
            bands[0] = xb0
            bias_sb = cpool.tile([C, 1], dt.float32, name="bias_sb")
            nc.sync.dma_start(bias_sb[:, :], bias[:, :])
            slope_sb = cpool.tile([C, 1], dt.float32, name="slope_sb")
            nc.sync.dma_start(slope_sb[:, :], slope[:, :])

            def row_ap(h, dx):
                """(128, 512) moving operand for source row h shifted by dx."""
                b, r = divmod(h, RB)
                return bands[b][:, r, PAD + dx : PAD + dx + W]

            for b in range(NBANDS):
                if b + 1 < NBANDS:
                    load_band(b + 1)  # prefetch
                h0 = b * RB
                psums = [psum_pool.tile([C, W], dt.float32, name="ps") for _ in range(RB)]
                # valid taps per row (rows 0/1 lose the dy=-2/-1 taps)
                valid = []
                for r in range(RB):
                    h = h0 + r
                    valid.append(
                        [t for t, (ky, kx) in enumerate(TAPS) if h + ky - PAD >= 0]
                    )
                # bank-major: all taps of a row consecutively, so each PSUM
                # bank stops ~13 MMs apart — its ACT drains while later rows
                # still matmul, and the next band never waits on bank release.
                # (Every matmul carries its own LDWEIGHTS either way, so tap
                # order costs nothing on the PE.)
                ob = out_pool.tile([C, RB, W], dt.float32, name="ob")
                for r in range(RB):
                    h = h0 + r
                    for t in valid[r]:
                        ky, kx = TAPS[t]
                        dy, dx = ky - PAD, kx - PAD
                        nc.tensor.matmul(
                            psums[r][:, :],
                            w_sb[:, t * C : (t + 1) * C],
                            row_ap(h + dy, dx),
                            start=(t == valid[r][0]),
                            stop=(t == valid[r][-1]),
                        )
                    nc.scalar.activation(
                        ob[:, r, :],
                        psums[r][:, :],
                        mybir.ActivationFunctionType.Prelu,
                        bias=bias_sb[:, 0:1],
                        scale=1.0,
                        alpha=slope_sb[:, 0:1],
                    )
                if b == NBANDS - 1:
                    # last band: drain output progressively behind the ACTs,
                    # finest chunks last so the final DMA is smallest
                    for r0, nr in ((0, 2), (2, 2), (4, 1), (5, 1), (6, 1), (7, 1)):
                        nc.sync.dma_start(
                            y[:, h0 + r0 : h0 + r0 + nr, :], ob[:, r0 : r0 + nr, :]
                        )
                else:
                    nc.sync.dma_start(y[:, h0 : h0 + RB, :], ob[:, :, :])
                if b - 1 in bands:
                    del bands[b - 1]
    nc.compile()
    return nc


def _get_nc(dtype_tag: str):
    if dtype_tag not in _CACHE:
        _CACHE[dtype_tag] = _build_bass(dtype_tag)
    return _CACHE[dtype_tag]


def _prep_weights(weight: np.ndarray, dtype_tag: str) -> np.ndarray:
    wm = weight.astype(np.float32) * _build_mask()
    wt = np.transpose(wm, (2, 3, 1, 0))  # (ky, kx, cin, cout)
    w_taps = np.concatenate([wt[ky, kx] for ky, kx in TAPS], axis=1)  # (128, 13*128)
    if dtype_tag == "bf16":
        import ml_dtypes

        return np.ascontiguousarray(w_taps).astype(ml_dtypes.bfloat16)
    if dtype_tag == "fp16":
        return np.ascontiguousarray(w_taps).astype(np.float16)
    return np.ascontiguousarray(w_taps)


def kernel(x, weight, bias, slope, dtype_tag="fp16", trace=False):
    from concourse.bass_utils import run_bass_kernel_spmd

    nc = _get_nc(dtype_tag)
    w_in = _prep_weights(np.asarray(weight), dtype_tag)
    bias_in = np.ascontiguousarray(np.asarray(bias, dtype=np.float32).reshape(C, 1))
    slope_in = np.ascontiguousarray(np.asarray(slope, dtype=np.float32).reshape(C, 1))
    x = np.asarray(x, dtype=np.float32)
    in_maps = [
        {
            "x": np.ascontiguousarray(x[i]),
            "w": w_in,
            "bias": bias_in,
            "slope": slope_in,
        }
        for i in range(B)
    ]
    res = run_bass_kernel_spmd(nc, in_maps, core_ids=list(range(B)), trace=trace)
    y = np.stack([res.results[i]["y"] for i in range(B)], axis=0)
    if trace:
        return y, res
    return y
